# revision 11
# baseline (speedup 1.0000x reference)
"""Trainium2 Bass kernel for nn_BilinearPairedLayer.

out[b,i,j,o] = celu(zl[b,i] @ fc_l_W^T + fc_l_b) @ W[o] @ celu(zr[b,j] @ fc_r_W^T + fc_r_b) + bb[o]

with context-3 pairing:
  zl = [x_l, shift_fwd(x_l,1), shift_bwd(x_l,1)]   (192 features)
  zr = [x_l, shift_bwd(x_r,1), shift_fwd(x_r,1)]   (faithful torch-source bug: x_l first)

Shapes: B=2, N=512, n_in=64, H=128, n_out=8 -> out [2,512,512,8] f32.

Sharding: 8 cores = (b in {0,1}) x (j-chunk in {0..3} of 128 columns).
Each core computes out[b, :, j0:j0+128, :] (as bf16; host upcasts).

Per-core dataflow (contraction dims pre-transposed onto partitions host-side).
Inputs ride both HWDGE rings so descriptor generation overlaps:
  sync:   D1w [128,384] bf16 (frT rows 0:64 / flT rows 64:128 overlay;
          each matmul's rhs is zero in the other side's rows), then
          Wt [128,1024] bf16 (WT[g, o*H+h] = bilinear_W[o,h,g])
  scalar: D1x [128,516] bf16 (xljT 0:128; xrhT 128:258; partition-0 rows
          @258/@386 = fc_r_b / fc_l_b for the K=1 bias matmuls; 514:516 =
          out-bias bb[p%8] f32 bitcast), then D1b [128,514] bf16
          (xlhT rows 64:128, zeros on top)

  0. PE warm-up: dummy matmuls on a memset tile while input DMAs fly, so
     HAM un-throttles (1.2->2.4 GHz) early; a dummy Exp pulls the ~1.3us
     ACT table load forward.
  1. fc biases are accumulated INTO PSUM by a K=1 rank-1 matmul
     (bias-row^T @ ones-row) that runs FIRST in each accumulation group,
     so celu needs no separate pre-add:
     celu = EXP (ACT, from psum) -> fused (-1,min 0) TS (DVE) -> max TT
     (DVE, psum operand), writing bf16.  The celu ops are emitted under
     tc.high_priority() so the Tile scheduler cannot hoist the v2 casts
     ahead of them.
  2. v2[h, j*8+o]: per o: WT_o.T @ hrT -> psum [h, o*128+j]; strided
     casts write the j-major/o-fast INTERLEAVED bf16 layout (both on
     ACT).  With this order, main-output partition p has o = p%8 for
     every chunk, so one shared [128,1] bias AP serves all evictions.
  3. TRANSPOSED main: psum[jo-block, i] = v2_c.T @ hlT, 8 matmuls N=512.
     PSUM bank map (8 banks, zero eviction-WAR stalls):
       ps_big B0-1: hr psum @[0:128], warm-up+hl psum @[512:1024],
                    then main c6/c7 (hr/hl long dead by then)
       pairA  B2-3: v2-og0 psum @[0:512], then main c0/c1 (WAW og0 cast)
       pairB  B4-5: v2-og1 psum @[0:512], then main c2/c3 (WAW og1 cast)
       pairC  B6-7: main c4/c5 (fresh)
  4. Evictions = one fused copy+bias+bf16-cast per group, [128,1024]
     two-bank ops where possible: ACT [c0c1][c4c5][c7], DVE [c2c3][c6];
     out DMAs per group: four on sync, the last on scalar right after
     ACT's c7 eviction.
  5. out DRAM [128, 4096] bf16: out_d[p, c*512+i] with p = jr*8+o,
     j = 16c+jr.  Host upcasts + unshuffles.

walrus's per-instruction HW structs carry at most ONE sync wait; a post-pass
splits multi-wait instructions into single-wait EventSemaphore predecessors.
"""

import numpy as np

import concourse.bass as bass
import concourse.mybir as mybir
import concourse.tile as tile
from concourse.bass_utils import run_bass_kernel_spmd

F32 = mybir.dt.float32
BF16 = mybir.dt.bfloat16

B = 2
N = 512
NIN = 64
H = 128
O = 8
JC = 128  # j-chunk per core
N_CORES = 8

# D1x packed-column offsets (bf16 elements)
_XLJ = 0              # xljT  [128]   (rows 0:64)
_XRH = 128            # xrhT  [130]   (rows 0:64)
_BRR = 258            # fc_r_b as a row on partition 0  [128]
_BLR = 386            # fc_l_b as a row on partition 0  [128]
_OBI = 514            # out-bias bb[p%8] per partition (f32 bitcast, 2 cols)
_D1XW = 516
_D1BW = 514           # xlhT (rows 64:128)

N_WARM = 11


def build_nc():
    nc = bass.Bass("TRN2")

    Dh = nc.dram_tensor("Dh", [128, 3 * H + _D1XW], BF16,
                        kind="ExternalInput")
    D1b = nc.dram_tensor("D1b", [128, _D1BW], BF16, kind="ExternalInput")
    Wt = nc.dram_tensor("Wt", [128, O * H], BF16, kind="ExternalInput")
    out_d = nc.dram_tensor("out", [128, O * N], BF16, kind="ExternalOutput")

    with tile.TileContext(nc) as tc:
        with (
            tc.tile_pool(name="persist", bufs=1) as pp,
            tc.tile_pool(name="psum", bufs=1, space="PSUM") as psp,
        ):
            Dh_sb = pp.tile([128, 3 * H + _D1XW], BF16, name="Dh_sb")
            D1b_sb = pp.tile([128, _D1BW], BF16, name="D1b_sb")
            W_sb = pp.tile([128, O * H], BF16, name="W_sb")
            warm_sb = pp.tile([128, 384], BF16, name="warm_sb")
            ones_sb = pp.tile([1, N], BF16, name="ones_sb")
            td = pp.tile([1, 2], F32, name="td")

            # ---- ALL inputs on the sync ring, ordered by first use:
            # the hot layer-1 block first (fast first-on-ring sem), then
            # D1b (hl matmuls), then Wt (v2).  The scalar ring carries
            # only the ACT table load, minimizing cross-queue contention.
            nc.sync.dma_start(Dh_sb[:], Dh[:])
            nc.sync.dma_start(D1b_sb[:], D1b[:])
            nc.sync.dma_start(W_sb[:], Wt[:])

            # ---- PE warm-up + early ACT table load ----
            nc.vector.memset(td[0:1, 0:1], 0.0)
            nc.vector.memset(warm_sb[:], 0.0)
            nc.vector.memset(ones_sb[:], 1.0)
            nc.scalar.activation(td[0:1, 1:2], td[0:1, 0:1],
                                 mybir.ActivationFunctionType.Exp)

            # PSUM map: separate tiles (PSUM dep tracking is coarse per
            # tile, so aliasing views would serialize unrelated ops).
            # Main matmul dsts reuse retired tiles; each reuse's implicit
            # tile-level WAR dep is one that the schedule satisfies anyway.
            ps_hr = psp.tile([128, JC], F32, name="ps_hr")
            ps_hl = psp.tile([128, N], F32, name="ps_hl")
            vAB = psp.tile([128, 1024], F32, name="vAB")
            pairA = psp.tile([128, 1024], F32, name="pairA")
            pairB = psp.tile([128, 1024], F32, name="pairB")
            ps_v0 = vAB[:, 0:512]
            ps_v1 = vAB[:, 512:1024]

            for _ in range(N_WARM):
                nc.tensor.matmul(
                    ps_hl[:, 0:256], warm_sb[:, 0:128], warm_sb[:, 128:384],
                    start=True, stop=True,
                )

            # ---- layer 1 matmuls; K=1 bias matmul runs FIRST ----
            nc.tensor.matmul(ps_hr[:], Dh_sb[0:1, 3 * H + _BRR:3 * H + _BRR + H],
                             ones_sb[0:1, 0:JC], start=True, stop=False)
            xo = 3 * H
            rhs_r = [
                Dh_sb[:, xo + _XLJ:xo + _XLJ + JC],          # x_l[j]
                Dh_sb[:, xo + _XRH + 2:xo + _XRH + 2 + JC],  # x_r[j+1]
                Dh_sb[:, xo + _XRH:xo + _XRH + JC],          # x_r[j-1]
            ]
            for c in range(3):
                nc.tensor.matmul(
                    ps_hr[:], Dh_sb[:, c * H:(c + 1) * H],
                    rhs_r[c], start=False, stop=(c == 2),
                )

            nc.tensor.matmul(ps_hl[:], Dh_sb[0:1, 3 * H + _BLR:3 * H + _BLR + H],
                             ones_sb[0:1, 0:N], start=True, stop=False)
            rhs_l = [
                D1b_sb[:, 1:1 + N],    # x_l[i]
                D1b_sb[:, 0:N],        # x_l[i-1] (fwd)
                D1b_sb[:, 2:2 + N],    # x_l[i+1] (bwd)
            ]
            for c in range(3):
                nc.tensor.matmul(
                    ps_hl[:], Dh_sb[:, c * H:(c + 1) * H],
                    rhs_l[c], start=False, stop=(c == 2),
                )

            # ---- hr celu: e (ACT) -> TS min (DVE) -> TT max (DVE) ----
            hrT = pp.tile([128, JC], BF16, name="hrT")
            e_r = pp.tile([128, JC], F32, name="e_r")
            with tc.high_priority():
                nc.scalar.activation(e_r[:], ps_hr[:],
                                     mybir.ActivationFunctionType.Exp)
                nc.vector.tensor_scalar(e_r[:], e_r[:], -1.0, 0.0,
                                        mybir.AluOpType.add,
                                        mybir.AluOpType.min)
                nc.vector.tensor_tensor(hrT[:], ps_hr[:], e_r[:],
                                        mybir.AluOpType.max)

            # ---- v2 matmuls: psum [h, (o,j)] per og group ----
            for og, ps_vo in ((0, ps_v0), (1, ps_v1)):
                for ol in range(4):
                    o = og * 4 + ol
                    nc.tensor.matmul(
                        ps_vo[:, ol * JC:(ol + 1) * JC],
                        W_sb[:, o * H:(o + 1) * H], hrT[:],
                        start=True, stop=True,
                    )

            # HAM keep-alive: bridge the PE idle gap between v2 and main
            # so the busy window stays unbroken and the un-throttle fires
            # before the main matmuls.
            for _ in range(3):
                nc.tensor.matmul(
                    pairA[:, 0:256], warm_sb[:, 0:128], warm_sb[:, 128:384],
                    start=True, stop=True,
                )

            # ---- hl celu (full width, priority-pinned before casts) ----
            hlT = pp.tile([128, N], BF16, name="hlT")
            e_l = pp.tile([128, N], F32, name="e_l")
            with tc.high_priority():
                nc.scalar.activation(e_l[:], ps_hl[:],
                                     mybir.ActivationFunctionType.Exp)
                nc.vector.tensor_scalar(e_l[:], e_l[:], -1.0, 0.0,
                                        mybir.AluOpType.add,
                                        mybir.AluOpType.min)
                nc.vector.tensor_tensor(hlT[:], ps_hl[:],
                                        e_l[:], mybir.AluOpType.max)

            # ---- v2 casts to interleaved bf16 layout: col = j*8 + o ----
            v2sb = pp.tile([128, O * H], BF16, name="v2sb")
            v2v = v2sb[:].rearrange("p (j o) -> p j o", o=8)
            nc.scalar.copy(
                v2v[:, :, 0:4],
                ps_v0[:].rearrange("p (o j) -> p j o", o=4))
            nc.scalar.copy(
                v2v[:, :, 4:8],
                ps_v1[:].rearrange("p (o j) -> p j o", o=4))

            # ---- main (transposed): psum[jo-block, i] = v2_c.T @ hlT ----
            # chunk c partition p -> j = 16c + p//8, o = p%8
            main_dst = [
                pairA[:, 0:512], pairA[:, 512:1024],
                pairB[:, 0:512], pairB[:, 512:1024],
                vAB[:, 0:512], vAB[:, 512:1024],   # WAR: after og casts
                ps_hl[:],                          # WAR: after hl celu
            ]
            for c in range(7):
                nc.tensor.matmul(
                    main_dst[c], v2sb[:, c * JC:(c + 1) * JC], hlT[:],
                    start=True, stop=True,
                )

            out_sb = pp.tile([128, O * N], BF16, name="out_sb")
            ob_ap = Dh_sb[:, 3 * H + _OBI:3 * H + _OBI + 2].bitcast(F32)

            def evict(eng, src, col0, col1):
                dst = out_sb[:, col0:col1]
                if eng is nc.scalar:
                    nc.scalar.activation(dst, src,
                                         mybir.ActivationFunctionType.Identity,
                                         bias=ob_ap, scale=1.0)
                else:
                    nc.vector.tensor_scalar_add(dst, src, ob_ap)

            evict(nc.scalar, pairA[:, 0:1024], 0, 1024)
            nc.sync.dma_start(out_d[:, 0:1024], out_sb[:, 0:1024])
            # c7 reuses pairA[0:512]; emitted AFTER evict01 so the WAR
            # dep (matmul waits the eviction's read) lands correctly.
            nc.tensor.matmul(
                pairA[:, 0:512], v2sb[:, 7 * JC:8 * JC], hlT[:],
                start=True, stop=True,
            )
            evict(nc.vector, pairB[:, 0:1024], 1024, 2048)
            evict(nc.scalar, vAB[:, 0:1024], 2048, 3072)
            nc.sync.dma_start(out_d[:, 1024:3072], out_sb[:, 1024:3072])
            evict(nc.vector, ps_hl[:], 3072, 3584)
            evict(nc.scalar, pairA[:, 0:512], 3584, 4096)
            nc.scalar.dma_start(out_d[:, 3072:4096], out_sb[:, 3072:4096])

    _legalize_waits(nc)
    return nc


def _legalize_waits(nc):
    """walrus's per-instruction HW structs carry at most ONE sync wait.
    Split any instruction with >1 on_wait into same-engine single-wait
    EventSemaphore predecessors (engine executes them in program order)."""
    n = 0
    for bb in nc.main_func.blocks:
        insts = list(bb.instructions)
        out = []
        for ins in insts:
            si = ins.sync_info
            waits = list(si.on_wait) if si and si.on_wait else []
            if len(waits) > 1:
                for w in waits[:-1]:
                    n += 1
                    out.append(mybir.InstEventSemaphore(
                        name=f"wait-split-{n}",
                        opcode="EventSemaphore",
                        engine=ins.engine,
                        ins=[], outs=[],
                        sync_info=mybir.SyncInfo(on_wait=[w], on_update=[]),
                    ))
                si.on_wait = [waits[-1]]
            out.append(ins)
        if n:
            bb.instructions = out
    return nc


_NC_CACHE = None


def _get_nc():
    global _NC_CACHE
    if _NC_CACHE is None:
        _NC_CACHE = build_nc()
    return _NC_CACHE


def _prep_core_inputs(x_l, x_r, fc_l_W, fc_l_b, fc_r_W, fc_r_b, bilinear_W, bilinear_b):
    """Host-side sharding: build the 8 per-core input dicts."""
    import ml_dtypes

    f32 = np.float32
    bf16 = ml_dtypes.bfloat16
    x_l = np.ascontiguousarray(x_l, f32)
    x_r = np.ascontiguousarray(x_r, f32)

    # WT[g, o*H + h] = W[o, h, g]
    WT = np.ascontiguousarray(
        np.asarray(bilinear_W, f32).transpose(2, 0, 1).reshape(128, O * H)
    ).astype(bf16)

    D1w = np.zeros((128, 3 * H), bf16)
    frW = np.asarray(fc_r_W, f32)
    flW = np.asarray(fc_l_W, f32)
    for c in range(3):
        D1w[:NIN, c * H:(c + 1) * H] = frW[:, c * NIN:(c + 1) * NIN].T.astype(bf16)
        D1w[NIN:, c * H:(c + 1) * H] = flW[:, c * NIN:(c + 1) * NIN].T.astype(bf16)

    D1x_c = np.zeros((128, _D1XW), bf16)
    D1x_c[0, _BRR:_BRR + H] = np.asarray(fc_r_b, f32).astype(bf16)
    D1x_c[0, _BLR:_BLR + H] = np.asarray(fc_l_b, f32).astype(bf16)
    obi = np.asarray(bilinear_b, f32)[np.arange(128) % O]  # bb[p%8]
    D1x_c.view(np.uint16)[:, _OBI:_OBI + 2] = obi.reshape(-1, 1).view('<u2')

    # D1b per batch: xlhT rows 64:128, col t = x_l[b, t-1]
    D1bs = []
    for b in range(B):
        D1b = np.zeros((128, _D1BW), bf16)
        D1b[NIN:, 1:1 + N] = x_l[b].T.astype(bf16)
        D1bs.append(D1b)

    in_maps = []
    for core in range(N_CORES):
        b, jg = core // 4, core % 4
        j0 = jg * JC
        D1x = D1x_c.copy()
        D1x[:NIN, _XLJ:_XLJ + JC] = x_l[b, j0:j0 + JC].T.astype(bf16)
        # xrhT: col t = x_r[b, j0-1+t], zero-padded at global edges
        lo = max(j0 - 1, 0)
        hi = min(j0 + JC + 1, N)
        D1x[:NIN, _XRH + lo - (j0 - 1):_XRH + hi - (j0 - 1)] = \
            x_r[b, lo:hi].T.astype(bf16)
        in_maps.append({
            "Dh": np.concatenate([D1w, D1x], axis=1),
            "D1b": D1bs[b],
            "Wt": WT,
        })
    return in_maps


def _run(inputs, trace=False, **kw):
    nc = _get_nc()
    in_maps = _prep_core_inputs(**inputs)
    res = run_bass_kernel_spmd(
        nc, in_maps, core_ids=list(range(N_CORES)), trace=trace, **kw)
    out = np.empty((B, N, N, O), np.float32)
    for core in range(N_CORES):
        b, jg = core // 4, core % 4
        j0 = jg * JC
        # device out: [p = jr*8+o, c*512 + i] -> out[i, 16c+jr, o]
        arr = np.asarray(res.results[core]["out"]).astype(np.float32)
        arr = arr.reshape(16, 8, 8, N)          # [jr, o, c, i]
        out[b, :, j0:j0 + JC, :] = \
            arr.transpose(3, 2, 0, 1).reshape(N, JC, O)
    return out, res


def kernel(**inputs):
    out, _ = _run(inputs, trace=False)
    return out


# revision 12
# speedup vs baseline: 1.1047x; 1.1047x over previous
"""Trainium2 Bass kernel for nn_BilinearPairedLayer.

out[b,i,j,o] = celu(zl[b,i] @ fc_l_W^T + fc_l_b) @ W[o] @ celu(zr[b,j] @ fc_r_W^T + fc_r_b) + bb[o]

with context-3 pairing:
  zl = [x_l, shift_fwd(x_l,1), shift_bwd(x_l,1)]   (192 features)
  zr = [x_l, shift_bwd(x_r,1), shift_fwd(x_r,1)]   (faithful torch-source bug: x_l first)

Shapes: B=2, N=512, n_in=64, H=128, n_out=8 -> out [2,512,512,8] f32.

Sharding: 8 cores = (b in {0,1}) x (j-chunk in {0..3} of 128 columns).
Each core computes out[b, :, j0:j0+128, :] (as bf16; host upcasts).

Per-core dataflow (contraction dims pre-transposed onto partitions host-side).
Inputs ride both HWDGE rings so descriptor generation overlaps:
  sync:   D1w [128,384] bf16 (frT rows 0:64 / flT rows 64:128 overlay;
          each matmul's rhs is zero in the other side's rows), then
          Wt [128,1024] bf16 (WT[g, o*H+h] = bilinear_W[o,h,g])
  scalar: D1x [128,516] bf16 (xljT 0:128; xrhT 128:258; partition-0 rows
          @258/@386 = fc_r_b / fc_l_b for the K=1 bias matmuls; 514:516 =
          out-bias bb[p%8] f32 bitcast), then D1b [128,514] bf16
          (xlhT rows 64:128, zeros on top)

  0. PE warm-up: dummy matmuls on a memset tile while input DMAs fly, so
     HAM un-throttles (1.2->2.4 GHz) early; a dummy Exp pulls the ~1.3us
     ACT table load forward.
  1. fc biases are accumulated INTO PSUM by a K=1 rank-1 matmul
     (bias-row^T @ ones-row) that runs FIRST in each accumulation group,
     so celu needs no separate pre-add:
     celu = EXP (ACT, from psum) -> fused (-1,min 0) TS (DVE) -> max TT
     (DVE, psum operand), writing bf16.  The celu ops are emitted under
     tc.high_priority() so the Tile scheduler cannot hoist the v2 casts
     ahead of them.
  2. v2[h, j*8+o]: per o: WT_o.T @ hrT -> psum [h, o*128+j]; strided
     casts write the j-major/o-fast INTERLEAVED bf16 layout (both on
     ACT).  With this order, main-output partition p has o = p%8 for
     every chunk, so one shared [128,1] bias AP serves all evictions.
  3. TRANSPOSED main: psum[jo-block, i] = v2_c.T @ hlT, 8 matmuls N=512.
     PSUM bank map (8 banks, zero eviction-WAR stalls):
       ps_big B0-1: hr psum @[0:128], warm-up+hl psum @[512:1024],
                    then main c6/c7 (hr/hl long dead by then)
       pairA  B2-3: v2-og0 psum @[0:512], then main c0/c1 (WAW og0 cast)
       pairB  B4-5: v2-og1 psum @[0:512], then main c2/c3 (WAW og1 cast)
       pairC  B6-7: main c4/c5 (fresh)
  4. Evictions = one fused copy+bias+bf16-cast per group, [128,1024]
     two-bank ops where possible: ACT [c0c1][c4c5][c7], DVE [c2c3][c6];
     out DMAs per group: four on sync, the last on scalar right after
     ACT's c7 eviction.
  5. out DRAM [128, 4096] bf16: out_d[p, c*512+i] with p = jr*8+o,
     j = 16c+jr.  Host upcasts + unshuffles.

walrus's per-instruction HW structs carry at most ONE sync wait; a post-pass
splits multi-wait instructions into single-wait EventSemaphore predecessors.
"""

import numpy as np

import concourse.bass as bass
import concourse.mybir as mybir
import concourse.tile as tile
from concourse.bass_utils import run_bass_kernel_spmd

F32 = mybir.dt.float32
BF16 = mybir.dt.bfloat16

B = 2
N = 512
NIN = 64
H = 128
O = 8
JC = 128  # j-chunk per core
N_CORES = 8

# D1x packed-column offsets (bf16 elements)
_XLJ = 0              # xljT  [128]   (rows 0:64)
_XRH = 128            # xrhT  [130]   (rows 0:64)
_BRR = 258            # fc_r_b as a row on partition 0  [128]
_BLR = 386            # fc_l_b as a row on partition 0  [128]
_OBI = 514            # out-bias bb[p%8] per partition (f32 bitcast, 2 cols)
_D1XW = 516
_D1BW = 514           # xlhT (rows 64:128)

N_WARM = 13


def build_nc():
    nc = bass.Bass("TRN2")

    Dh = nc.dram_tensor("Dh", [128, 3 * H + _D1XW + _D1BW], BF16,
                        kind="ExternalInput")
    Wt = nc.dram_tensor("Wt", [128, O * H], BF16, kind="ExternalInput")
    out_d = nc.dram_tensor("out", [128, O * N], BF16, kind="ExternalOutput")

    with tile.TileContext(nc) as tc:
        with (
            tc.tile_pool(name="persist", bufs=1) as pp,
            tc.tile_pool(name="psum", bufs=1, space="PSUM") as psp,
        ):
            Dh_sb = pp.tile([128, 3 * H + _D1XW + _D1BW], BF16, name="Dh_sb")
            W_sb = pp.tile([128, O * H], BF16, name="W_sb")
            warm_sb = pp.tile([128, 384], BF16, name="warm_sb")
            ones_sb = pp.tile([1, N], BF16, name="ones_sb")
            td = pp.tile([1, 2], F32, name="td")

            # ---- ALL inputs on the sync ring: the hot layer-1 block
            # (incl. D1b) first so it drains at full rate with a fast
            # first-on-ring sem; Wt behind it.  The scalar ring carries
            # only the ACT table load, minimizing cross-queue contention.
            nc.sync.dma_start(Dh_sb[:], Dh[:])
            nc.sync.dma_start(W_sb[:], Wt[:])

            # ---- PE warm-up + early ACT table load ----
            nc.vector.memset(td[0:1, 0:1], 0.0)
            nc.vector.memset(warm_sb[:], 0.0)
            nc.vector.memset(ones_sb[:], 1.0)
            nc.scalar.activation(td[0:1, 1:2], td[0:1, 0:1],
                                 mybir.ActivationFunctionType.Exp)

            # PSUM map: separate tiles (PSUM dep tracking is coarse per
            # tile, so aliasing views would serialize unrelated ops).
            # Main matmul dsts reuse retired tiles; each reuse's implicit
            # tile-level WAR dep is one that the schedule satisfies anyway.
            ps_hr = psp.tile([128, JC], F32, name="ps_hr")
            ps_hl = psp.tile([128, N], F32, name="ps_hl")
            vAB = psp.tile([128, 1024], F32, name="vAB")
            pairA = psp.tile([128, 1024], F32, name="pairA")
            pairB = psp.tile([128, 1024], F32, name="pairB")
            ps_v0 = vAB[:, 0:512]
            ps_v1 = vAB[:, 512:1024]

            for _ in range(N_WARM):
                nc.tensor.matmul(
                    ps_hl[:, 0:256], warm_sb[:, 0:128], warm_sb[:, 128:384],
                    start=True, stop=True,
                )

            # ---- layer 1 matmuls; K=1 bias matmul runs FIRST ----
            nc.tensor.matmul(ps_hr[:], Dh_sb[0:1, 3 * H + _BRR:3 * H + _BRR + H],
                             ones_sb[0:1, 0:JC], start=True, stop=False)
            xo = 3 * H
            rhs_r = [
                Dh_sb[:, xo + _XLJ:xo + _XLJ + JC],          # x_l[j]
                Dh_sb[:, xo + _XRH + 2:xo + _XRH + 2 + JC],  # x_r[j+1]
                Dh_sb[:, xo + _XRH:xo + _XRH + JC],          # x_r[j-1]
            ]
            for c in range(3):
                nc.tensor.matmul(
                    ps_hr[:], Dh_sb[:, c * H:(c + 1) * H],
                    rhs_r[c], start=False, stop=(c == 2),
                )

            nc.tensor.matmul(ps_hl[:], Dh_sb[0:1, 3 * H + _BLR:3 * H + _BLR + H],
                             ones_sb[0:1, 0:N], start=True, stop=False)
            xb = 3 * H + _D1XW
            rhs_l = [
                Dh_sb[:, xb + 1:xb + 1 + N],    # x_l[i]
                Dh_sb[:, xb + 0:xb + N],        # x_l[i-1] (fwd)
                Dh_sb[:, xb + 2:xb + 2 + N],    # x_l[i+1] (bwd)
            ]
            for c in range(3):
                nc.tensor.matmul(
                    ps_hl[:], Dh_sb[:, c * H:(c + 1) * H],
                    rhs_l[c], start=False, stop=(c == 2),
                )

            # ---- hr celu: e (ACT) -> TS min (DVE) -> TT max (DVE) ----
            hrT = pp.tile([128, JC], BF16, name="hrT")
            e_r = pp.tile([128, JC], F32, name="e_r")
            with tc.high_priority():
                nc.scalar.activation(e_r[:], ps_hr[:],
                                     mybir.ActivationFunctionType.Exp)
                nc.vector.tensor_scalar(e_r[:], e_r[:], -1.0, 0.0,
                                        mybir.AluOpType.add,
                                        mybir.AluOpType.min)
                nc.vector.tensor_tensor(hrT[:], ps_hr[:], e_r[:],
                                        mybir.AluOpType.max)

            # ---- v2 matmuls: psum [h, (o,j)] per og group ----
            for og, ps_vo in ((0, ps_v0), (1, ps_v1)):
                for ol in range(4):
                    o = og * 4 + ol
                    nc.tensor.matmul(
                        ps_vo[:, ol * JC:(ol + 1) * JC],
                        W_sb[:, o * H:(o + 1) * H], hrT[:],
                        start=True, stop=True,
                    )

            # HAM keep-alive: bridge the PE idle gap between v2 and main
            # so the busy window stays unbroken and the un-throttle fires
            # before the main matmuls.
            for _ in range(3):
                nc.tensor.matmul(
                    pairA[:, 0:256], warm_sb[:, 0:128], warm_sb[:, 128:384],
                    start=True, stop=True,
                )

            # ---- hl celu (full width, priority-pinned before casts) ----
            hlT = pp.tile([128, N], BF16, name="hlT")
            e_l = pp.tile([128, N], F32, name="e_l")
            with tc.high_priority():
                nc.scalar.activation(e_l[:], ps_hl[:],
                                     mybir.ActivationFunctionType.Exp)
                nc.vector.tensor_scalar(e_l[:], e_l[:], -1.0, 0.0,
                                        mybir.AluOpType.add,
                                        mybir.AluOpType.min)
                nc.vector.tensor_tensor(hlT[:], ps_hl[:],
                                        e_l[:], mybir.AluOpType.max)

            # ---- v2 casts to interleaved bf16 layout: col = j*8 + o ----
            v2sb = pp.tile([128, O * H], BF16, name="v2sb")
            v2v = v2sb[:].rearrange("p (j o) -> p j o", o=8)
            nc.scalar.copy(
                v2v[:, :, 0:4],
                ps_v0[:].rearrange("p (o j) -> p j o", o=4))
            nc.scalar.copy(
                v2v[:, :, 4:8],
                ps_v1[:].rearrange("p (o j) -> p j o", o=4))

            # ---- main (transposed): psum[jo-block, i] = v2_c.T @ hlT ----
            # chunk c partition p -> j = 16c + p//8, o = p%8
            main_dst = [
                pairA[:, 0:512], pairA[:, 512:1024],
                pairB[:, 0:512], pairB[:, 512:1024],
                vAB[:, 0:512], vAB[:, 512:1024],   # WAR: after og casts
                ps_hl[:],                          # WAR: after hl celu
            ]
            for c in range(7):
                nc.tensor.matmul(
                    main_dst[c], v2sb[:, c * JC:(c + 1) * JC], hlT[:],
                    start=True, stop=True,
                )

            out_sb = pp.tile([128, O * N], BF16, name="out_sb")
            ob_ap = Dh_sb[:, 3 * H + _OBI:3 * H + _OBI + 2].bitcast(F32)

            def evict(eng, src, col0, col1):
                dst = out_sb[:, col0:col1]
                if eng is nc.scalar:
                    nc.scalar.activation(dst, src,
                                         mybir.ActivationFunctionType.Identity,
                                         bias=ob_ap, scale=1.0)
                else:
                    nc.vector.tensor_scalar_add(dst, src, ob_ap)

            evict(nc.scalar, pairA[:, 0:1024], 0, 1024)
            nc.sync.dma_start(out_d[:, 0:1024], out_sb[:, 0:1024])
            # c7 reuses pairA[0:512]; emitted AFTER evict01 so the WAR
            # dep (matmul waits the eviction's read) lands correctly.
            nc.tensor.matmul(
                pairA[:, 0:512], v2sb[:, 7 * JC:8 * JC], hlT[:],
                start=True, stop=True,
            )
            evict(nc.vector, pairB[:, 0:1024], 1024, 2048)
            evict(nc.scalar, vAB[:, 0:1024], 2048, 3072)
            nc.sync.dma_start(out_d[:, 1024:3072], out_sb[:, 1024:3072])
            evict(nc.vector, ps_hl[:], 3072, 3584)
            evict(nc.scalar, pairA[:, 0:512], 3584, 4096)
            nc.scalar.dma_start(out_d[:, 3072:4096], out_sb[:, 3072:4096])

    _legalize_waits(nc)
    return nc


def _legalize_waits(nc):
    """walrus's per-instruction HW structs carry at most ONE sync wait.
    Split any instruction with >1 on_wait into same-engine single-wait
    EventSemaphore predecessors (engine executes them in program order)."""
    n = 0
    for bb in nc.main_func.blocks:
        insts = list(bb.instructions)
        out = []
        for ins in insts:
            si = ins.sync_info
            waits = list(si.on_wait) if si and si.on_wait else []
            if len(waits) > 1:
                for w in waits[:-1]:
                    n += 1
                    out.append(mybir.InstEventSemaphore(
                        name=f"wait-split-{n}",
                        opcode="EventSemaphore",
                        engine=ins.engine,
                        ins=[], outs=[],
                        sync_info=mybir.SyncInfo(on_wait=[w], on_update=[]),
                    ))
                si.on_wait = [waits[-1]]
            out.append(ins)
        if n:
            bb.instructions = out
    return nc


_NC_CACHE = None


def _get_nc():
    global _NC_CACHE
    if _NC_CACHE is None:
        _NC_CACHE = build_nc()
    return _NC_CACHE


def _prep_core_inputs(x_l, x_r, fc_l_W, fc_l_b, fc_r_W, fc_r_b, bilinear_W, bilinear_b):
    """Host-side sharding: build the 8 per-core input dicts."""
    import ml_dtypes

    f32 = np.float32
    bf16 = ml_dtypes.bfloat16
    x_l = np.ascontiguousarray(x_l, f32)
    x_r = np.ascontiguousarray(x_r, f32)

    # WT[g, o*H + h] = W[o, h, g]
    WT = np.ascontiguousarray(
        np.asarray(bilinear_W, f32).transpose(2, 0, 1).reshape(128, O * H)
    ).astype(bf16)

    D1w = np.zeros((128, 3 * H), bf16)
    frW = np.asarray(fc_r_W, f32)
    flW = np.asarray(fc_l_W, f32)
    for c in range(3):
        D1w[:NIN, c * H:(c + 1) * H] = frW[:, c * NIN:(c + 1) * NIN].T.astype(bf16)
        D1w[NIN:, c * H:(c + 1) * H] = flW[:, c * NIN:(c + 1) * NIN].T.astype(bf16)

    D1x_c = np.zeros((128, _D1XW), bf16)
    D1x_c[0, _BRR:_BRR + H] = np.asarray(fc_r_b, f32).astype(bf16)
    D1x_c[0, _BLR:_BLR + H] = np.asarray(fc_l_b, f32).astype(bf16)
    obi = np.asarray(bilinear_b, f32)[np.arange(128) % O]  # bb[p%8]
    D1x_c.view(np.uint16)[:, _OBI:_OBI + 2] = obi.reshape(-1, 1).view('<u2')

    # D1b per batch: xlhT rows 64:128, col t = x_l[b, t-1]
    D1bs = []
    for b in range(B):
        D1b = np.zeros((128, _D1BW), bf16)
        D1b[NIN:, 1:1 + N] = x_l[b].T.astype(bf16)
        D1bs.append(D1b)

    in_maps = []
    for core in range(N_CORES):
        b, jg = core // 4, core % 4
        j0 = jg * JC
        D1x = D1x_c.copy()
        D1x[:NIN, _XLJ:_XLJ + JC] = x_l[b, j0:j0 + JC].T.astype(bf16)
        # xrhT: col t = x_r[b, j0-1+t], zero-padded at global edges
        lo = max(j0 - 1, 0)
        hi = min(j0 + JC + 1, N)
        D1x[:NIN, _XRH + lo - (j0 - 1):_XRH + hi - (j0 - 1)] = \
            x_r[b, lo:hi].T.astype(bf16)
        in_maps.append({
            "Dh": np.concatenate([D1w, D1x, D1bs[b]], axis=1),
            "Wt": WT,
        })
    return in_maps


def _run(inputs, trace=False, **kw):
    nc = _get_nc()
    in_maps = _prep_core_inputs(**inputs)
    res = run_bass_kernel_spmd(
        nc, in_maps, core_ids=list(range(N_CORES)), trace=trace, **kw)
    out = np.empty((B, N, N, O), np.float32)
    for core in range(N_CORES):
        b, jg = core // 4, core % 4
        j0 = jg * JC
        # device out: [p = jr*8+o, c*512 + i] -> out[i, 16c+jr, o]
        arr = np.asarray(res.results[core]["out"]).astype(np.float32)
        arr = arr.reshape(16, 8, 8, N)          # [jr, o, c, i]
        out[b, :, j0:j0 + JC, :] = \
            arr.transpose(3, 2, 0, 1).reshape(N, JC, O)
    return out, res


def kernel(**inputs):
    out, _ = _run(inputs, trace=False)
    return out
